# revision 10
# baseline (speedup 1.0000x reference)
"""Trainium2 Bass kernel for DEMA (Holt's linear trend) decomposition.

reference:  ma = DEMA(x) along time (alpha=0.3, beta=0.1), res = x - ma,
            x: [32, 4096, 128] fp32, returns (res, ma).

Approach: the DEMA is a 2x2 linear recurrence v_t = A v_{t-1} + c x_t with
spectral radius sqrt(0.7) ~ 0.837, so the impulse response decays below 1e-10
within 128 steps.  The scan therefore collapses to a banded lower-triangular
matmul (FIR) over time:  with 128-step time blocks,
    ma_blk[i] = W0 @ x_blk[i] + W1 @ x_blk[i-1]
with constant 128x128 Toeplitz coefficient blocks (W0 lower-triangular band,
W1 the band crossing the block boundary).  Blocks 0/1 get rank-2 corrections
carrying the s0/b0 initial-condition terms.  This maps onto the TensorEngine:
contraction over source-time (partitions), (batch x channel) on the moving
free dim.

Performance notes (the kernel is DMA-roofline bound at ~360 GB/s/core):
  - outputs are written as fp16 (host upcasts): halves store traffic, and the
    quantization error (~4e-4 rel) is far inside the 2e-2 tolerance.
  - matmuls run as float32r (full-precision fp32 data, 1 cycle/row PE mode
    for moving dims >= 256) so the TensorEngine stays off the critical path.
  - x / res / ma live in DRAM in the SBUF-friendly [P, blk, b, c] layout
    (host pre/post-transposes), making every DMA one fully-contiguous
    descriptor block per partition: 7 load + 18 store DMAs total.
  - all DMAs issue from SP in FIFO order [xg0, wts, corr, xg1..xg4,
    stores...]: weights land right after the first x group so compute starts
    at ~9.4us, and every store chunk is ready well before the engines drain
    the loads -- the DMA mutex runs gap-free start to finish.
  - per-core budget: 2.33us lead-in + 46.97us DMA-busy + 1.74us tail
    = 51.04us (loads 8.39MB fp32 + weights 133KB + stores 8.39MB fp16
    at the 360B/ns DMA ceiling).

Sharding: batch 32 -> 4 per core across 8 cores, no communication.
"""

import numpy as np

ALPHA = 0.3
BETA = 0.1
P = 128          # time block = partition dim
B, T, C = 32, 4096, 128
NCORES = 8
BL = B // NCORES  # local batch = 4
NB = T // P       # 32 time blocks
FREE = BL * C     # matmul moving free dim = 512


def _build_coeffs():
    """Return ([128, 256], [2, 256]) fp32 = (concat([W0T, W1T], 1),
    rank-2 initial-condition corrections), transposed for matmul lhsT."""
    dt = np.float64
    A = np.array([[1 - ALPHA, 1 - ALPHA],
                  [-ALPHA * BETA, BETA * (1 - ALPHA) + 1 - BETA]], dtype=dt)
    c = np.array([ALPHA, ALPHA * BETA], dtype=dt)
    n = 2 * P
    Apow = np.empty((n + 1, 2, 2), dtype=dt)
    Apow[0] = np.eye(2)
    for j in range(1, n + 1):
        Apow[j] = Apow[j - 1] @ A
    w = np.einsum('jab,b->ja', Apow, c)[:, 0]  # w[j] = (A^j c)[0]

    # Exact coefficient rows for the first two blocks (initial conditions:
    # s0 = x0, b0 = x1 - x0 fold into columns 0 and 1).
    G2 = np.zeros((n, n), dtype=dt)
    G2[0, 0] = 1.0
    for t in range(1, n):
        G2[t, 2:t + 1] = w[t - 2::-1][:max(t - 1, 0)]
        G2[t, 1] = w[t - 1] + Apow[t][0, 1]
        G2[t, 0] = Apow[t][0, 0] - Apow[t][0, 1]

    r = np.arange(P)
    jmat = r[:, None] - r[None, :]
    W0 = np.where(jmat >= 0, w[np.clip(jmat, 0, n)], 0.0)
    W1 = w[P + jmat]
    M00 = G2[0:P, 0:P]
    M10 = G2[P:2 * P, 0:P]
    # M00/M10 differ from W0/W1 only in columns 0-1 (the s0/b0 initial
    # condition terms) -> rank-2 corrections applied with K=2 matmuls.
    wts = np.concatenate([W0.T, W1.T], axis=1)
    corr = np.concatenate([(M00 - W0).T[0:2], (M10 - W1).T[0:2]], axis=1)
    return (np.ascontiguousarray(wts.astype(np.float32)),
            np.ascontiguousarray(corr.astype(np.float32)))


def _fix_multi_waits(nc):
    """The walrus build in this container rejects instructions with more than
    one sync wait ("Too many sync wait commands" in setupSyncWait).  Move all
    but the last wait of any multi-wait instruction onto freshly inserted
    same-engine NoOps placed immediately before it (same sequencer, earlier
    program order => semantically equivalent)."""
    import concourse.mybir as mybir

    for f in nc.m.functions:
        for bb in f.blocks:
            insts = bb.instructions
            if not any(
                i.sync_info and i.sync_info.on_wait and len(i.sync_info.on_wait) > 1
                for i in insts
            ):
                continue
            new = []
            for inst in insts:
                si = inst.sync_info
                waits = list(si.on_wait) if si and si.on_wait else []
                if len(waits) > 1:
                    for k, w in enumerate(waits[:-1]):
                        new.append(mybir.InstNoOp(
                            name=f"{inst.name}-wsplit{k}",
                            sync_info=mybir.SyncInfo(on_wait=[w], on_update=[]),
                            bass_nofuse=True,
                            engine=inst.engine,
                        ))
                    si.on_wait = [waits[-1]]
                    inst.sync_info = si
                new.append(inst)
            bb.instructions = new


GS = [8, 8, 8, 4, 4]               # x-load group sizes (blocks per load DMA)
SCS = [2, 4, 4, 4, 4, 4, 4, 4, 2]  # store chunk sizes (blocks per store DMA)


def build_bass():
    """Build the per-core Bass module (SPMD: same NEFF on all 8 cores)."""
    import concourse.bass as bass
    import concourse.mybir as mybir
    from concourse.tile import TileContext

    assert sum(GS) == NB and sum(SCS) == NB
    f32 = mybir.dt.float32
    f32r = mybir.dt.float32r
    f16 = mybir.dt.float16

    nc = bass.Bass()
    # DRAM layout [p(time-within-block), blk, b, c]: matches the SBUF tile
    # layout exactly, so every DMA is one contiguous run per partition.
    x = nc.dram_tensor("x", [P, NB, BL, C], f32r, kind="ExternalInput")
    wts = nc.dram_tensor("wts", [P, 2 * P], f32r, kind="ExternalInput")
    corr = nc.dram_tensor("corr", [2, 2 * P], f32r, kind="ExternalInput")
    res = nc.dram_tensor("res", [P, NB, BL, C], f16, kind="ExternalOutput")
    ma = nc.dram_tensor("ma", [P, NB, BL, C], f16, kind="ExternalOutput")

    with TileContext(nc) as tc:
        with (
            tc.tile_pool(name="wpool", bufs=1) as wpool,
            tc.tile_pool(name="xpool", bufs=len(GS)) as xpool,
            tc.tile_pool(name="mapool", bufs=6) as mapool,
            tc.tile_pool(name="respool", bufs=6) as respool,
            tc.tile_pool(name="psum", bufs=6, space="PSUM") as psumpool,
        ):
            # All DMAs issue from SP (fastest HWDGE path, in-order FIFO):
            # compute engines never stall behind a DMA issue sequence.
            xsec = {}  # global block index -> SBUF section [P, FREE]
            wt = ct = None
            blk0 = 0
            for gi, gsz in enumerate(GS):
                xg = xpool.tile([P, gsz * FREE], f32r, tag="xg")
                nc.sync.dma_start(
                    out=xg[:],
                    in_=x[:, blk0:blk0 + gsz, :, :],
                )
                for j in range(gsz):
                    xsec[blk0 + j] = xg[:, j * FREE:(j + 1) * FREE]
                blk0 += gsz
                if gi == 0:
                    # Weights ride SP between the first and second x group so
                    # the DMA-engine FIFO order is [xg0, wts, corr, xg1, ...]:
                    # compute can start the moment xg0+wts are resident.
                    wt = wpool.tile([P, 2 * P], f32r)
                    nc.sync.dma_start(out=wt[:], in_=wts[:])
                    ct = wpool.tile([2, 2 * P], f32r)
                    nc.sync.dma_start(out=ct[:], in_=corr[:])

            w0r = wt[:, 0 * P:1 * P]
            w1r = wt[:, 1 * P:2 * P]
            c0r = ct[:, 0:P]
            c1r = ct[:, P:2 * P]

            sci = 0      # store chunk index
            jc = 0       # block index within store chunk
            mac = resc = None
            for i in range(NB):
                xc = xsec[i]
                if jc == 0:
                    mac = mapool.tile([P, SCS[sci] * FREE], f16, tag="mac")
                    resc = respool.tile([P, SCS[sci] * FREE], f16, tag="resc")
                ps = psumpool.tile([P, FREE], f32)
                if i == 0:
                    nc.tensor.matmul(ps[:], w0r, xc, start=True, stop=False)
                    nc.tensor.matmul(ps[:], c0r, xc[0:2, :], start=False, stop=True)
                elif i == 1:
                    xp = xsec[0]
                    nc.tensor.matmul(ps[:], w0r, xc, start=True, stop=False)
                    nc.tensor.matmul(ps[:], w1r, xp, start=False, stop=False)
                    nc.tensor.matmul(ps[:], c1r, xp[0:2, :], start=False, stop=True)
                else:
                    xp = xsec[i - 1]
                    nc.tensor.matmul(ps[:], w0r, xc, start=True, stop=False)
                    nc.tensor.matmul(ps[:], w1r, xp, start=False, stop=True)
                ma_sec = mac[:, jc * FREE:(jc + 1) * FREE]
                res_sec = resc[:, jc * FREE:(jc + 1) * FREE]
                nc.scalar.copy(out=ma_sec, in_=ps[:])
                nc.vector.tensor_sub(out=res_sec, in0=xc.bitcast(f32), in1=ps[:])
                jc += 1
                if jc == SCS[sci]:
                    blks = slice(i + 1 - SCS[sci], i + 1)
                    nc.sync.dma_start(out=ma[:, blks, :, :], in_=mac[:])
                    nc.sync.dma_start(out=res[:, blks, :, :], in_=resc[:])
                    sci += 1
                    jc = 0
    _fix_multi_waits(nc)
    return nc


_CACHE = {}


def kernel(x):
    from concourse.bass_utils import run_bass_kernel_spmd

    x = np.ascontiguousarray(np.asarray(x), dtype=np.float32)
    assert x.shape == (B, T, C), x.shape

    if "nc" not in _CACHE:
        _CACHE["nc"] = build_bass()
        _CACHE["wts"], _CACHE["corr"] = _build_coeffs()
    nc = _CACHE["nc"]

    # [B, T, C] -> per-core [P, NB, BL, C]
    xt = x.reshape(NCORES, BL, NB, P, C).transpose(0, 3, 2, 1, 4)
    in_maps = [
        {"x": np.ascontiguousarray(xt[i]),
         "wts": _CACHE["wts"], "corr": _CACHE["corr"]}
        for i in range(NCORES)
    ]
    r = run_bass_kernel_spmd(nc, in_maps, core_ids=list(range(NCORES)))

    def unshard(name):
        # per-core [P, NB, BL, C] f16 -> [B, T, C] f32
        parts = [r.results[i][name].transpose(2, 1, 0, 3).reshape(BL, T, C)
                 for i in range(NCORES)]
        return np.concatenate(parts, axis=0).astype(np.float32)

    return unshard("res"), unshard("ma")


# revision 14
# speedup vs baseline: 1.0068x; 1.0068x over previous
"""Trainium2 Bass kernel for DEMA (Holt's linear trend) decomposition.

reference:  ma = DEMA(x) along time (alpha=0.3, beta=0.1), res = x - ma,
            x: [32, 4096, 128] fp32, returns (res, ma).

Approach: the DEMA is a 2x2 linear recurrence v_t = A v_{t-1} + c x_t with
spectral radius sqrt(0.7) ~ 0.837, so the impulse response decays below 1e-10
within 128 steps.  The scan therefore collapses to a banded lower-triangular
matmul (FIR) over time:  with 128-step time blocks,
    ma_blk[i] = W0 @ x_blk[i] + W1 @ x_blk[i-1]
with constant 128x128 Toeplitz coefficient blocks (W0 lower-triangular band,
W1 the band crossing the block boundary).  Blocks 0/1 get rank-2 corrections
carrying the s0/b0 initial-condition terms.  This maps onto the TensorEngine:
contraction over source-time (partitions), (batch x channel) on the moving
free dim.

Performance notes (the kernel is DMA-roofline bound at ~360 GB/s/core):
  - outputs are written as fp16 (host upcasts): halves store traffic, and the
    quantization error (~4e-4 rel) is far inside the 2e-2 tolerance.
  - matmuls run as float32r (full-precision fp32 data, 1 cycle/row PE mode
    for moving dims >= 256) so the TensorEngine stays off the critical path.
  - x / res / ma live in DRAM in the SBUF-friendly [P, blk, b, c] layout
    (host pre/post-transposes), making every DMA one fully-contiguous
    descriptor block per partition: 7 load + 18 store DMAs total.
  - all DMAs issue from SP in FIFO order [xg0, wts, corr, xg1..xg4,
    stores...]: weights land right after the first x group so compute starts
    at ~9.4us, and every store chunk is ready well before the engines drain
    the loads -- the DMA mutex runs gap-free start to finish.
  - per-core budget: 2.33us lead-in + 46.97us DMA-busy + 1.74us tail
    = 51.04us (loads 8.39MB fp32 + weights 133KB + stores 8.39MB fp16
    at the 360B/ns DMA ceiling).

Sharding: batch 32 -> 4 per core across 8 cores, no communication.
"""

import numpy as np

ALPHA = 0.3
BETA = 0.1
P = 128          # time block = partition dim
B, T, C = 32, 4096, 128
NCORES = 8
BL = B // NCORES  # local batch = 4
NB = T // P       # 32 time blocks
FREE = BL * C     # matmul moving free dim = 512


def _build_coeffs():
    """Return (u [1, 512], v [128, 2], corr [2, 256]) fp32.

    The FIR weight block wt[k, m] = w[m - k] (k partition, m in [0, 256):
    cols 0-127 = W0^T, cols 128-255 = W1^T) is EXACTLY rank-2 before the
    causal mask: with complex eigenvalues r e^{+-iw} of the companion
    matrix,  w[j] = r^j (p cos wj + q sin wj),  so
        w[m-k] = u1[m] v1[k] + u2[m] v2[k]
        u1[m] = r^m cos(wm)          v1[k] = r^-k (p cos wk - q sin wk)
        u2[m] = r^m sin(wm)          v2[k] = r^-k (p sin wk + q cos wk)
    The device rebuilds wt from these 3 KB of vectors (one K=1 matmul to
    broadcast u + two tensor_scalar + add + causal affine_select), so the
    128 KB weight matrix never crosses HBM.  corr carries the exact rank-2
    initial-condition corrections for blocks 0/1 as before."""
    dt = np.float64
    A = np.array([[1 - ALPHA, 1 - ALPHA],
                  [-ALPHA * BETA, BETA * (1 - ALPHA) + 1 - BETA]], dtype=dt)
    c = np.array([ALPHA, ALPHA * BETA], dtype=dt)
    n = 2 * P
    Apow = np.empty((n + 1, 2, 2), dtype=dt)
    Apow[0] = np.eye(2)
    for j in range(1, n + 1):
        Apow[j] = Apow[j - 1] @ A
    w = np.einsum('jab,b->ja', Apow, c)[:, 0]  # w[j] = (A^j c)[0]

    # Exact coefficient rows for the first two blocks (initial conditions:
    # s0 = x0, b0 = x1 - x0 fold into columns 0 and 1).
    G2 = np.zeros((n, n), dtype=dt)
    G2[0, 0] = 1.0
    for t in range(1, n):
        G2[t, 2:t + 1] = w[t - 2::-1][:max(t - 1, 0)]
        G2[t, 1] = w[t - 1] + Apow[t][0, 1]
        G2[t, 0] = Apow[t][0, 0] - Apow[t][0, 1]

    r = np.arange(P)
    jmat = r[:, None] - r[None, :]
    W0 = np.where(jmat >= 0, w[np.clip(jmat, 0, n)], 0.0)
    W1 = w[P + jmat]
    M00 = G2[0:P, 0:P]
    M10 = G2[P:2 * P, 0:P]
    # M00/M10 differ from W0/W1 only in columns 0-1 (the s0/b0 initial
    # condition terms) -> rank-2 corrections applied with K=2 matmuls.
    corr = np.concatenate([(M00 - W0).T[0:2], (M10 - W1).T[0:2]], axis=1)

    tr, det = A[0, 0] + A[1, 1], A[0, 0] * A[1, 1] - A[0, 1] * A[1, 0]
    rr = np.sqrt(det)
    om = np.arctan2(np.sqrt(4 * det - tr * tr) / 2, tr / 2)
    p = w[0]
    q = (w[1] / rr - p * np.cos(om)) / np.sin(om)
    m = np.arange(2 * P, dtype=dt)
    k = np.arange(P, dtype=dt)
    u = np.concatenate([rr**m * np.cos(om * m), rr**m * np.sin(om * m)])
    v = np.stack([rr**-k * (p * np.cos(om * k) - q * np.sin(om * k)),
                  rr**-k * (p * np.sin(om * k) + q * np.cos(om * k))], axis=1)
    return (np.ascontiguousarray(u[None, :].astype(np.float32)),
            np.ascontiguousarray(v.astype(np.float32)),
            np.ascontiguousarray(corr.astype(np.float32)))


def _fix_multi_waits(nc):
    """The walrus build in this container rejects instructions with more than
    one sync wait ("Too many sync wait commands" in setupSyncWait).  Move all
    but the last wait of any multi-wait instruction onto freshly inserted
    same-engine NoOps placed immediately before it (same sequencer, earlier
    program order => semantically equivalent)."""
    import concourse.mybir as mybir

    for f in nc.m.functions:
        for bb in f.blocks:
            insts = bb.instructions
            if not any(
                i.sync_info and i.sync_info.on_wait and len(i.sync_info.on_wait) > 1
                for i in insts
            ):
                continue
            new = []
            for inst in insts:
                si = inst.sync_info
                waits = list(si.on_wait) if si and si.on_wait else []
                if len(waits) > 1:
                    for k, w in enumerate(waits[:-1]):
                        new.append(mybir.InstNoOp(
                            name=f"{inst.name}-wsplit{k}",
                            sync_info=mybir.SyncInfo(on_wait=[w], on_update=[]),
                            bass_nofuse=True,
                            engine=inst.engine,
                        ))
                    si.on_wait = [waits[-1]]
                    inst.sync_info = si
                new.append(inst)
            bb.instructions = new


GS = [8, 8, 8, 4, 4]               # x-load group sizes (blocks per load DMA)
SCS = [2, 4, 4, 4, 4, 4, 4, 4, 2]  # store chunk sizes (blocks per store DMA)


def build_bass():
    """Build the per-core Bass module (SPMD: same NEFF on all 8 cores)."""
    import concourse.bass as bass
    import concourse.mybir as mybir
    from concourse.tile import TileContext

    assert sum(GS) == NB and sum(SCS) == NB
    f32 = mybir.dt.float32
    f32r = mybir.dt.float32r
    f16 = mybir.dt.float16

    nc = bass.Bass()
    # DRAM layout [p(time-within-block), blk, b, c]: matches the SBUF tile
    # layout exactly, so every DMA is one contiguous run per partition.
    x = nc.dram_tensor("x", [P, NB, BL, C], f32r, kind="ExternalInput")
    uvec = nc.dram_tensor("uvec", [1, 4 * P], f32, kind="ExternalInput")
    vvec = nc.dram_tensor("vvec", [P, 2], f32, kind="ExternalInput")
    corr = nc.dram_tensor("corr", [2, 2 * P], f32r, kind="ExternalInput")
    res = nc.dram_tensor("res", [P, NB, BL, C], f16, kind="ExternalOutput")
    ma = nc.dram_tensor("ma", [P, NB, BL, C], f16, kind="ExternalOutput")

    with TileContext(nc) as tc:
        with (
            tc.tile_pool(name="wpool", bufs=1) as wpool,
            tc.tile_pool(name="xpool", bufs=len(GS)) as xpool,
            tc.tile_pool(name="mapool", bufs=6) as mapool,
            tc.tile_pool(name="respool", bufs=6) as respool,
            tc.tile_pool(name="psum", bufs=6, space="PSUM") as psumpool,
            tc.tile_pool(name="wgenps", bufs=1, space="PSUM") as wgenpool,
        ):
            # All DMAs issue from SP (fastest HWDGE path, in-order FIFO):
            # compute engines never stall behind a DMA issue sequence.
            xsec = {}  # global block index -> SBUF section [P, FREE]
            wt = ct = None
            blk0 = 0
            for gi, gsz in enumerate(GS):
                xg = xpool.tile([P, gsz * FREE], f32r, tag="xg")
                nc.sync.dma_start(
                    out=xg[:],
                    in_=x[:, blk0:blk0 + gsz, :, :],
                )
                for j in range(gsz):
                    xsec[blk0 + j] = xg[:, j * FREE:(j + 1) * FREE]
                blk0 += gsz
                if gi == 0:
                    # The tiny weight-ingredient DMAs ride SP between the
                    # first and second x group: FIFO order [xg0, u, v, corr,
                    # xg1, ...], so weight generation and compute start the
                    # moment xg0 lands.
                    usb = wpool.tile([1, 4 * P], f32)
                    nc.sync.dma_start(out=usb[:], in_=uvec[:])
                    vsb = wpool.tile([P, 2], f32)
                    nc.sync.dma_start(out=vsb[:], in_=vvec[:])
                    ct = wpool.tile([2, 2 * P], f32r)
                    nc.sync.dma_start(out=ct[:], in_=corr[:])

            # Rebuild wt[k, m] = w[m-k] (causal-masked) from the rank-2
            # vectors: broadcast u across partitions with a K=1 ones matmul,
            # scale each 256-wide half by its per-partition v column, add,
            # then zero the j = m-k < 0 triangle.  ~3.5us of otherwise-idle
            # PE/DVE time replaces 131 KB (364 ns) of weight DMA.
            ones = wpool.tile([1, P], f32)
            nc.vector.memset(ones[:], 1.0)
            psu = wgenpool.tile([P, 4 * P], f32)
            nc.tensor.matmul(psu[:], ones[:], usb[:], start=True, stop=True)
            wtmp = wpool.tile([P, 2 * P], f32)
            nc.vector.tensor_scalar(wtmp[:], psu[:, 0:2 * P], vsb[:, 0:1],
                                    None, mybir.AluOpType.mult)
            wt = wpool.tile([P, 2 * P], f32r)
            nc.vector.tensor_scalar(wt[:], psu[:, 2 * P:4 * P], vsb[:, 1:2],
                                    None, mybir.AluOpType.mult)
            nc.vector.tensor_add(out=wtmp[:], in0=wtmp[:],
                                 in1=wt[:].bitcast(f32))
            nc.gpsimd.affine_select(wt[:], wtmp[:], [[1, 2 * P]],
                                    mybir.AluOpType.is_ge, 0.0,
                                    base=0, channel_multiplier=-1)
            w0r = wt[:, 0 * P:1 * P]
            w1r = wt[:, 1 * P:2 * P]
            c0r = ct[:, 0:P]
            c1r = ct[:, P:2 * P]

            sci = 0      # store chunk index
            jc = 0       # block index within store chunk
            mac = resc = None
            for i in range(NB):
                xc = xsec[i]
                if jc == 0:
                    mac = mapool.tile([P, SCS[sci] * FREE], f16, tag="mac")
                    resc = respool.tile([P, SCS[sci] * FREE], f16, tag="resc")
                ps = psumpool.tile([P, FREE], f32)
                if i == 0:
                    nc.tensor.matmul(ps[:], w0r, xc, start=True, stop=False)
                    nc.tensor.matmul(ps[:], c0r, xc[0:2, :], start=False, stop=True)
                elif i == 1:
                    xp = xsec[0]
                    nc.tensor.matmul(ps[:], w0r, xc, start=True, stop=False)
                    nc.tensor.matmul(ps[:], w1r, xp, start=False, stop=False)
                    nc.tensor.matmul(ps[:], c1r, xp[0:2, :], start=False, stop=True)
                else:
                    xp = xsec[i - 1]
                    nc.tensor.matmul(ps[:], w0r, xc, start=True, stop=False)
                    nc.tensor.matmul(ps[:], w1r, xp, start=False, stop=True)
                ma_sec = mac[:, jc * FREE:(jc + 1) * FREE]
                res_sec = resc[:, jc * FREE:(jc + 1) * FREE]
                nc.scalar.copy(out=ma_sec, in_=ps[:])
                nc.vector.tensor_sub(out=res_sec, in0=xc.bitcast(f32), in1=ps[:])
                jc += 1
                if jc == SCS[sci]:
                    blks = slice(i + 1 - SCS[sci], i + 1)
                    nc.sync.dma_start(out=ma[:, blks, :, :], in_=mac[:])
                    nc.sync.dma_start(out=res[:, blks, :, :], in_=resc[:])
                    sci += 1
                    jc = 0
    _fix_multi_waits(nc)
    return nc


_CACHE = {}


def kernel(x):
    from concourse.bass_utils import run_bass_kernel_spmd

    x = np.ascontiguousarray(np.asarray(x), dtype=np.float32)
    assert x.shape == (B, T, C), x.shape

    if "nc" not in _CACHE:
        _CACHE["nc"] = build_bass()
        _CACHE["u"], _CACHE["v"], _CACHE["corr"] = _build_coeffs()
    nc = _CACHE["nc"]

    # [B, T, C] -> per-core [P, NB, BL, C]
    xt = x.reshape(NCORES, BL, NB, P, C).transpose(0, 3, 2, 1, 4)
    in_maps = [
        {"x": np.ascontiguousarray(xt[i]), "uvec": _CACHE["u"],
         "vvec": _CACHE["v"], "corr": _CACHE["corr"]}
        for i in range(NCORES)
    ]
    r = run_bass_kernel_spmd(nc, in_maps, core_ids=list(range(NCORES)))

    def unshard(name):
        # per-core [P, NB, BL, C] f16 -> [B, T, C] f32
        parts = [r.results[i][name].transpose(2, 1, 0, 3).reshape(BL, T, C)
                 for i in range(NCORES)]
        return np.concatenate(parts, axis=0).astype(np.float32)

    return unshard("res"), unshard("ma")


# revision 16
# speedup vs baseline: 1.0079x; 1.0010x over previous
"""Trainium2 Bass kernel for DEMA (Holt's linear trend) decomposition.

reference:  ma = DEMA(x) along time (alpha=0.3, beta=0.1), res = x - ma,
            x: [32, 4096, 128] fp32, returns (res, ma).

Approach: the DEMA is a 2x2 linear recurrence v_t = A v_{t-1} + c x_t with
spectral radius sqrt(0.7) ~ 0.837, so the impulse response decays below 1e-10
within 128 steps.  The scan therefore collapses to a banded lower-triangular
matmul (FIR) over time:  with 128-step time blocks,
    ma_blk[i] = W0 @ x_blk[i] + W1 @ x_blk[i-1]
with constant 128x128 Toeplitz coefficient blocks (W0 lower-triangular band,
W1 the band crossing the block boundary).  Blocks 0/1 get rank-2 corrections
carrying the s0/b0 initial-condition terms.  This maps onto the TensorEngine:
contraction over source-time (partitions), (batch x channel) on the moving
free dim.

Performance notes (the kernel is DMA-roofline bound at ~360 GB/s/core):
  - outputs are written as fp16 (host upcasts): halves store traffic, and the
    quantization error (~4e-4 rel) is far inside the 2e-2 tolerance.
  - matmuls run as float32r (full-precision fp32 data, 1 cycle/row PE mode
    for moving dims >= 256) so the TensorEngine stays off the critical path.
  - x / res / ma live in DRAM in the SBUF-friendly [P, blk, b, c] layout
    (host pre/post-transposes), making every DMA one fully-contiguous
    descriptor block per partition: 8 load + 18 store DMAs total.
  - the 128 KB FIR weight matrix never crosses HBM: w[m-k] is rank-2 in
    (m, k), so 5 KB of host-computed vectors + ~3us of otherwise-idle
    PE/DVE/Pool time rebuild it on-core (see _build_coeffs).
  - all DMAs issue from SP in FIFO order [xg0, u, v, corr, xg1..xg4,
    stores...]: weight generation overlaps the xg1 load, and every store
    chunk is ready well before the engines drain the loads -- the DMA
    mutex runs gap-free start to finish.
  - per-core budget: 2.33us lead-in + 46.62us DMA-busy + 1.69us tail
    = 50.64us, with the DMA stream within ~30ns of the mandatory-byte
    floor (8.39MB fp32 in + 8.39MB fp16 out at the 360B/ns ceiling).

Sharding: batch 32 -> 4 per core across 8 cores, no communication.
"""

import numpy as np

ALPHA = 0.3
BETA = 0.1
P = 128          # time block = partition dim
B, T, C = 32, 4096, 128
NCORES = 8
BL = B // NCORES  # local batch = 4
NB = T // P       # 32 time blocks
FREE = BL * C     # matmul moving free dim = 512


def _build_coeffs():
    """Return (u [1, 512], v [128, 2], corr [2, 256]) fp32.

    The FIR weight block wt[k, m] = w[m - k] (k partition, m in [0, 256):
    cols 0-127 = W0^T, cols 128-255 = W1^T) is EXACTLY rank-2 before the
    causal mask: with complex eigenvalues r e^{+-iw} of the companion
    matrix,  w[j] = r^j (p cos wj + q sin wj),  so
        w[m-k] = u1[m] v1[k] + u2[m] v2[k]
        u1[m] = r^m cos(wm)          v1[k] = r^-k (p cos wk - q sin wk)
        u2[m] = r^m sin(wm)          v2[k] = r^-k (p sin wk + q cos wk)
    The device rebuilds wt from these 3 KB of vectors (one K=1 matmul to
    broadcast u + two tensor_scalar + add + causal affine_select), so the
    128 KB weight matrix never crosses HBM.  corr carries the exact rank-2
    initial-condition corrections for blocks 0/1 as before."""
    dt = np.float64
    A = np.array([[1 - ALPHA, 1 - ALPHA],
                  [-ALPHA * BETA, BETA * (1 - ALPHA) + 1 - BETA]], dtype=dt)
    c = np.array([ALPHA, ALPHA * BETA], dtype=dt)
    n = 2 * P
    Apow = np.empty((n + 1, 2, 2), dtype=dt)
    Apow[0] = np.eye(2)
    for j in range(1, n + 1):
        Apow[j] = Apow[j - 1] @ A
    w = np.einsum('jab,b->ja', Apow, c)[:, 0]  # w[j] = (A^j c)[0]

    # Exact coefficient rows for the first two blocks (initial conditions:
    # s0 = x0, b0 = x1 - x0 fold into columns 0 and 1).
    G2 = np.zeros((n, n), dtype=dt)
    G2[0, 0] = 1.0
    for t in range(1, n):
        G2[t, 2:t + 1] = w[t - 2::-1][:max(t - 1, 0)]
        G2[t, 1] = w[t - 1] + Apow[t][0, 1]
        G2[t, 0] = Apow[t][0, 0] - Apow[t][0, 1]

    r = np.arange(P)
    jmat = r[:, None] - r[None, :]
    W0 = np.where(jmat >= 0, w[np.clip(jmat, 0, n)], 0.0)
    W1 = w[P + jmat]
    M00 = G2[0:P, 0:P]
    M10 = G2[P:2 * P, 0:P]
    # M00/M10 differ from W0/W1 only in columns 0-1 (the s0/b0 initial
    # condition terms) -> rank-2 corrections applied with K=2 matmuls.
    corr = np.concatenate([(M00 - W0).T[0:2], (M10 - W1).T[0:2]], axis=1)

    tr, det = A[0, 0] + A[1, 1], A[0, 0] * A[1, 1] - A[0, 1] * A[1, 0]
    rr = np.sqrt(det)
    om = np.arctan2(np.sqrt(4 * det - tr * tr) / 2, tr / 2)
    p = w[0]
    q = (w[1] / rr - p * np.cos(om)) / np.sin(om)
    m = np.arange(2 * P, dtype=dt)
    k = np.arange(P, dtype=dt)
    u = np.concatenate([rr**m * np.cos(om * m), rr**m * np.sin(om * m)])
    v = np.stack([rr**-k * (p * np.cos(om * k) - q * np.sin(om * k)),
                  rr**-k * (p * np.sin(om * k) + q * np.cos(om * k))], axis=0)
    return (np.ascontiguousarray(u[None, :].astype(np.float32)),
            np.ascontiguousarray(v.astype(np.float32)),
            np.ascontiguousarray(corr.astype(np.float32)))


def _fix_multi_waits(nc):
    """The walrus build in this container rejects instructions with more than
    one sync wait ("Too many sync wait commands" in setupSyncWait).  Move all
    but the last wait of any multi-wait instruction onto freshly inserted
    same-engine NoOps placed immediately before it (same sequencer, earlier
    program order => semantically equivalent)."""
    import concourse.mybir as mybir

    for f in nc.m.functions:
        for bb in f.blocks:
            insts = bb.instructions
            if not any(
                i.sync_info and i.sync_info.on_wait and len(i.sync_info.on_wait) > 1
                for i in insts
            ):
                continue
            new = []
            for inst in insts:
                si = inst.sync_info
                waits = list(si.on_wait) if si and si.on_wait else []
                if len(waits) > 1:
                    for k, w in enumerate(waits[:-1]):
                        new.append(mybir.InstNoOp(
                            name=f"{inst.name}-wsplit{k}",
                            sync_info=mybir.SyncInfo(on_wait=[w], on_update=[]),
                            bass_nofuse=True,
                            engine=inst.engine,
                        ))
                    si.on_wait = [waits[-1]]
                    inst.sync_info = si
                new.append(inst)
            bb.instructions = new


GS = [8, 8, 8, 4, 4]               # x-load group sizes (blocks per load DMA)
SCS = [2, 4, 4, 4, 4, 4, 4, 4, 2]  # store chunk sizes (blocks per store DMA)


def build_bass():
    """Build the per-core Bass module (SPMD: same NEFF on all 8 cores)."""
    import concourse.bass as bass
    import concourse.mybir as mybir
    from concourse.tile import TileContext

    assert sum(GS) == NB and sum(SCS) == NB
    f32 = mybir.dt.float32
    f32r = mybir.dt.float32r
    f16 = mybir.dt.float16

    nc = bass.Bass()
    # DRAM layout [p(time-within-block), blk, b, c]: matches the SBUF tile
    # layout exactly, so every DMA is one contiguous run per partition.
    x = nc.dram_tensor("x", [P, NB, BL, C], f32r, kind="ExternalInput")
    uvec = nc.dram_tensor("uvec", [1, 4 * P], f32, kind="ExternalInput")
    vvec = nc.dram_tensor("vvec", [2, P], f32, kind="ExternalInput")
    corr = nc.dram_tensor("corr", [2, 2 * P], f32r, kind="ExternalInput")
    res = nc.dram_tensor("res", [P, NB, BL, C], f16, kind="ExternalOutput")
    ma = nc.dram_tensor("ma", [P, NB, BL, C], f16, kind="ExternalOutput")

    with TileContext(nc) as tc:
        with (
            tc.tile_pool(name="wpool", bufs=1) as wpool,
            tc.tile_pool(name="xpool", bufs=len(GS)) as xpool,
            tc.tile_pool(name="mapool", bufs=6) as mapool,
            tc.tile_pool(name="respool", bufs=6) as respool,
            tc.tile_pool(name="psum", bufs=6, space="PSUM") as psumpool,
            tc.tile_pool(name="wgenps", bufs=1, space="PSUM") as wgenpool,
        ):
            # All DMAs issue from SP (fastest HWDGE path, in-order FIFO):
            # compute engines never stall behind a DMA issue sequence.
            xsec = {}  # global block index -> SBUF section [P, FREE]
            wt = ct = None
            blk0 = 0
            for gi, gsz in enumerate(GS):
                xg = xpool.tile([P, gsz * FREE], f32r, tag="xg")
                nc.sync.dma_start(
                    out=xg[:],
                    in_=x[:, blk0:blk0 + gsz, :, :],
                )
                for j in range(gsz):
                    xsec[blk0 + j] = xg[:, j * FREE:(j + 1) * FREE]
                blk0 += gsz
                if gi == 0:
                    # The tiny weight-ingredient DMAs ride SP between the
                    # first and second x group: FIFO order [xg0, u, v, corr,
                    # xg1, ...], so weight generation and compute start the
                    # moment xg0 lands.
                    usb = wpool.tile([1, 4 * P], f32)
                    nc.sync.dma_start(out=usb[:], in_=uvec[:])
                    vrows = wpool.tile([2, P], f32)
                    nc.sync.dma_start(out=vrows[:], in_=vvec[:])
                    ct = wpool.tile([2, 2 * P], f32r)
                    nc.sync.dma_start(out=ct[:], in_=corr[:])

            # Rebuild wt[k, m] = w[m-k] (causal-masked) from the rank-2
            # vectors: broadcast u across partitions with a K=1 ones matmul,
            # scale each 256-wide half by its per-partition v column, add,
            # then zero the j = m-k < 0 triangle.  ~3.5us of otherwise-idle
            # PE/DVE time replaces 131 KB (364 ns) of weight DMA.
            ones = wpool.tile([1, P], f32)
            nc.vector.memset(ones[:], 1.0)
            # v arrives as two 128-wide rows (512 B DMA descriptors); PE
            # transposes it to the per-partition [P, 2] layout tensor_scalar
            # needs.  The 2x2 identity for the transpose is built in place.
            id2 = wpool.tile([2, 2], f32)
            nc.vector.memset(id2[:], 1.0)
            nc.gpsimd.affine_select(id2[:], id2[:], [[1, 2]],
                                    mybir.AluOpType.is_equal, 0.0,
                                    base=0, channel_multiplier=-1)
            psv = wgenpool.tile([P, 2], f32)
            nc.tensor.transpose(psv[:], vrows[:], id2[:])
            vsb = wpool.tile([P, 2], f32)
            nc.vector.tensor_copy(out=vsb[:], in_=psv[:])
            psu = wgenpool.tile([P, 4 * P], f32)
            nc.tensor.matmul(psu[:], ones[:], usb[:], start=True, stop=True)
            wtmp = wpool.tile([P, 2 * P], f32)
            nc.vector.tensor_scalar(wtmp[:], psu[:, 0:2 * P], vsb[:, 0:1],
                                    None, mybir.AluOpType.mult)
            wt = wpool.tile([P, 2 * P], f32r)
            nc.vector.tensor_scalar(wt[:], psu[:, 2 * P:4 * P], vsb[:, 1:2],
                                    None, mybir.AluOpType.mult)
            nc.vector.tensor_add(out=wtmp[:], in0=wtmp[:],
                                 in1=wt[:].bitcast(f32))
            nc.gpsimd.affine_select(wt[:], wtmp[:], [[1, 2 * P]],
                                    mybir.AluOpType.is_ge, 0.0,
                                    base=0, channel_multiplier=-1)
            w0r = wt[:, 0 * P:1 * P]
            w1r = wt[:, 1 * P:2 * P]
            c0r = ct[:, 0:P]
            c1r = ct[:, P:2 * P]

            sci = 0      # store chunk index
            jc = 0       # block index within store chunk
            mac = resc = None
            for i in range(NB):
                xc = xsec[i]
                if jc == 0:
                    mac = mapool.tile([P, SCS[sci] * FREE], f16, tag="mac")
                    resc = respool.tile([P, SCS[sci] * FREE], f16, tag="resc")
                ps = psumpool.tile([P, FREE], f32)
                if i == 0:
                    nc.tensor.matmul(ps[:], w0r, xc, start=True, stop=False)
                    nc.tensor.matmul(ps[:], c0r, xc[0:2, :], start=False, stop=True)
                elif i == 1:
                    xp = xsec[0]
                    nc.tensor.matmul(ps[:], w0r, xc, start=True, stop=False)
                    nc.tensor.matmul(ps[:], w1r, xp, start=False, stop=False)
                    nc.tensor.matmul(ps[:], c1r, xp[0:2, :], start=False, stop=True)
                else:
                    xp = xsec[i - 1]
                    nc.tensor.matmul(ps[:], w0r, xc, start=True, stop=False)
                    nc.tensor.matmul(ps[:], w1r, xp, start=False, stop=True)
                ma_sec = mac[:, jc * FREE:(jc + 1) * FREE]
                res_sec = resc[:, jc * FREE:(jc + 1) * FREE]
                nc.scalar.copy(out=ma_sec, in_=ps[:])
                nc.vector.tensor_sub(out=res_sec, in0=xc.bitcast(f32), in1=ps[:])
                jc += 1
                if jc == SCS[sci]:
                    blks = slice(i + 1 - SCS[sci], i + 1)
                    nc.sync.dma_start(out=ma[:, blks, :, :], in_=mac[:])
                    nc.sync.dma_start(out=res[:, blks, :, :], in_=resc[:])
                    sci += 1
                    jc = 0
    _fix_multi_waits(nc)
    return nc


_CACHE = {}


def kernel(x):
    from concourse.bass_utils import run_bass_kernel_spmd

    x = np.ascontiguousarray(np.asarray(x), dtype=np.float32)
    assert x.shape == (B, T, C), x.shape

    if "nc" not in _CACHE:
        _CACHE["nc"] = build_bass()
        _CACHE["u"], _CACHE["v"], _CACHE["corr"] = _build_coeffs()
    nc = _CACHE["nc"]

    # [B, T, C] -> per-core [P, NB, BL, C]
    xt = x.reshape(NCORES, BL, NB, P, C).transpose(0, 3, 2, 1, 4)
    in_maps = [
        {"x": np.ascontiguousarray(xt[i]), "uvec": _CACHE["u"],
         "vvec": _CACHE["v"], "corr": _CACHE["corr"]}
        for i in range(NCORES)
    ]
    r = run_bass_kernel_spmd(nc, in_maps, core_ids=list(range(NCORES)))

    def unshard(name):
        # per-core [P, NB, BL, C] f16 -> [B, T, C] f32
        parts = [r.results[i][name].transpose(2, 1, 0, 3).reshape(BL, T, C)
                 for i in range(NCORES)]
        return np.concatenate(parts, axis=0).astype(np.float32)

    return unshard("res"), unshard("ma")
